# revision 20
# baseline (speedup 1.0000x reference)
"""CrossAttention on 8 TRN2 NeuronCores (tensor-parallel over heads).

Reference computation (B=4, N=2048, DIM=1024, 16 heads, head_dim=64):
    qkv = x @ Wqkv.T + bqkv ; q, k = split(qkv)  (v unused)
    attn = softmax(q @ k.T * scale) ; out = attn @ split_heads(context)
    return merge_heads(out) @ Wout.T + bout

Sharding: core c owns heads {2c, 2c+1}. Each core computes q/k
projections for its heads (full sequence), head-parallel attention with
context slices as values, then half-batch AllToAlls re-shard from
head-parallel to row-parallel so the output projection runs locally.
Row ownership is interleaved at 128-row granularity (core c owns rows
[c*128:(c+1)*128] and [1024+c*128:...] of every batch) so each
half-batch collective delivers work to every core immediately.

Engine plan (per 512-query group): the PE emits the 16 score matmul
pairs (two heads on disjoint PE row groups run concurrently) finely
interleaved with the value matmuls of four-kc-older tiles plus
qkproj / outproj "filler" chunks, so the in-order PE stream never waits
long on softmax. Exp is split across engines: 11/16 tiles on ScalarE
(ActivationFunctionType.Exp) and 5/16 on VectorE via a bf16 Schraudolph
bit-trick (x*A+B -> int16 -> bitcast bf16; ~1.8% RMS per element, ~1.1%
end-to-end after softmax). The softmax denominator comes from an
all-ones 65th value column; normalization is reciprocal_approx_fast +
DMA broadcast + one VectorE multiply reading PSUM directly.

DMA queues: bulk weight/activation loads issue from the Scalar queue,
latency-critical small transfers (denominator bounce, a2a stores,
outproj slices) from the SP queue, and gpsimd runs only collectives.
"""
import numpy as np
import ml_dtypes

import concourse.bass as bass
import concourse.mybir as mybir
import concourse.tile as tile
from concourse import bacc
from concourse.bass_utils import run_bass_kernel_spmd

BF16 = ml_dtypes.bfloat16
F32 = mybir.dt.float32
BF = mybir.dt.bfloat16
I16 = mybir.dt.int16

NC = 8            # cores
B = 4             # batch
N = 2048          # sequence
DIM = 1024
NH = 16           # heads total
HD = 64           # head dim
HPC = NH // NC    # heads per core = 2
SCALE = HD ** -0.5
BN = B * N        # 8192 tokens
RPB = N // NC     # rows per (core, batch) = 256 (two 128-row chunks)
KC = DIM // 128   # contraction chunks for projections = 8
NKC = N // 128    # key chunks per batch = 16
CW = HD + 1       # value width incl. ones column = 65
NG = 4            # 512-query groups per batch
QTAG = 3          # live generations of q/k tiles

# bf16 Schraudolph exp: bitcast(int16(x*SCALE*184.665 + 16248.5))
SCH_A = 184.66502435 * SCALE
SCH_B = 16248.5


def build(DVE_KC=(3, 6, 9, 12, 15), LAG=8, FILL=True):
    nc = bacc.Bacc("TRN2", target_bir_lowering=False, debug=False,
                   num_devices=NC)

    xT = nc.dram_tensor("xT", [DIM, BN], BF, kind="ExternalInput")
    wqkT = nc.dram_tensor("wqkT", [DIM, 2 * 128], BF, kind="ExternalInput")
    bqk = nc.dram_tensor("bqk", [2 * 128, 1], F32, kind="ExternalInput")
    ctxa = nc.dram_tensor("ctxa", [B, HPC, 128, NKC * CW], BF,
                          kind="ExternalInput")
    woutT = nc.dram_tensor("woutT", [DIM, DIM], BF, kind="ExternalInput")
    boutb = nc.dram_tensor("boutb", [128, DIM], F32, kind="ExternalInput")
    out = nc.dram_tensor("out", [B * RPB, DIM], F32, kind="ExternalOutput")

    # half-batch AllToAll buffers: phase (b, p) covers query rows
    # [p*1024:(p+1)*1024]; chunk j holds its rows [j*128:(j+1)*128]
    a2a_in = [[nc.dram_tensor(f"a2a_in{b}_{p}", [NC, 128, 128], BF)
               for p in range(2)] for b in range(B)]
    a2a_out = [[nc.dram_tensor(f"a2a_out{b}_{p}", [NC, 128, 128], BF)
                for p in range(2)] for b in range(B)]

    with tile.TileContext(nc) as tc:
        with tc.tile_pool(name="const", bufs=1) as const, \
             tc.tile_pool(name="qk", bufs=1) as qkpool, \
             tc.tile_pool(name="xt", bufs=12) as xtpool, \
             tc.tile_pool(name="xtf", bufs=24) as xtfpool, \
             tc.tile_pool(name="pt", bufs=18) as ptpool, \
             tc.tile_pool(name="r1", bufs=2) as r1pool, \
             tc.tile_pool(name="rb", bufs=4) as rbpool, \
             tc.tile_pool(name="ho", bufs=4) as hopool, \
             tc.tile_pool(name="sl", bufs=3) as slpool, \
             tc.tile_pool(name="ob", bufs=2) as obpool, \
             tc.tile_pool(name="pss", bufs=2, space="PSUM") as pss_pool, \
             tc.tile_pool(name="psm", bufs=4, space="PSUM") as psm_pool:

            wqk_sb = []
            for kc in range(KC):
                t = const.tile([128, 256], BF, tag=f"wqk{kc}")
                nc.sync.dma_start(out=t[:],
                                  in_=wqkT[kc * 128:(kc + 1) * 128, :])
                wqk_sb.append(t)
            bq_sb = []
            for fb in range(2):
                t = const.tile([128, 1], F32, tag=f"bq{fb}")
                nc.sync.dma_start(out=t[:],
                                  in_=bqk[fb * 128:(fb + 1) * 128, :])
                bq_sb.append(t)

            wout_sb = []
            bout_sb = const.tile([128, DIM], F32, tag="bout")
            ctx_sb = {}
            qk_tiles = {}
            xt_tiles = {}
            sl_tiles = {}

            def load_out_consts():
                for fc in range(KC):
                    t = const.tile([128, DIM], BF, tag=f"wout{fc}",
                                   name=f"wout{fc}")

                    def ld(t=t, fc=fc):
                        nc.sync.dma_start(
                            out=t[:], in_=woutT[fc * 128:(fc + 1) * 128, :])
                    bulk_dmas.append(ld)
                    wout_sb.append(t)
                bulk_dmas.append(
                    lambda: nc.sync.dma_start(out=bout_sb[:], in_=boutb[:]))

            def load_ctx(b, defer=False):
                for h in range(HPC):
                    t = const.tile([128, NKC * CW], BF, tag=f"ctx{b}{h}",
                                   name=f"ctx{b}_{h}")

                    def ld(t=t, b=b, h=h):
                        nc.sync.dma_start(out=t[:], in_=ctxa[b, h, :, :])
                    if defer:
                        bulk_dmas.append(ld)
                    else:
                        ld()
                    ctx_sb[b, h] = t

            bulk_dmas = []

            def flush_bulk(n):
                for _ in range(min(n, len(bulk_dmas))):
                    bulk_dmas.pop(0)()

            def prefetch_x(b, fine=False, defer=False):
                qT = qkpool.tile([128, N], BF, tag=f"qT{b % QTAG}",
                                 name=f"qT{b}")
                kT = qkpool.tile([128, N], BF, tag=f"kT{b % QTAG}",
                                 name=f"kT{b}")
                qk_tiles[b] = (qT, kT)
                if fine:
                    xts = {}
                    for t in range(4):
                        for kc in range(KC):
                            xt = xtfpool.tile([128, 512], BF, tag="xtf",
                                              name=f"xtf{b}_{kc}_{t}")
                            nc.sync.dma_start(
                                out=xt[:],
                                in_=xT[kc * 128:(kc + 1) * 128,
                                       b * N + t * 512:b * N + (t + 1) * 512])
                            xts[kc, t] = xt
                else:
                    xts = []
                    for kc in range(KC):
                        xt = xtpool.tile([128, N], BF, tag="xt",
                                         name=f"xtb{b}_{kc}")

                        def ld(xt=xt, kc=kc, b=b):
                            nc.sync.dma_start(
                                out=xt[:], in_=xT[kc * 128:(kc + 1) * 128,
                                                  b * N:(b + 1) * N])
                        if defer:
                            bulk_dmas.append(ld)
                        else:
                            ld()
                        xts.append(xt)
                xt_tiles[b] = xts

            def xslice(b, kc, t):
                x = xt_tiles[b]
                if isinstance(x, dict):
                    return x[kc, t][:]
                return x[kc][:, t * 512:(t + 1) * 512]

            def qk_half(b, t, fb):
                """Project one 512-token chunk for q (fb=0) or k (fb=1)."""
                qT, kT = qk_tiles[b]
                dst = kT if fb else qT
                ps = psm_pool.tile([128, 512], F32, tag="psm",
                                   name=f"psq{b}_{t}_{fb}")
                for kc in range(KC):
                    nc.tensor.matmul(
                        ps[:], wqk_sb[kc][:, fb * 128:(fb + 1) * 128],
                        xslice(b, kc, t),
                        start=(kc == 0), stop=(kc == KC - 1))
                nc.vector.tensor_scalar_add(
                    dst[:, t * 512:(t + 1) * 512], ps[:], bq_sb[fb][:])

            def out_sl(b, rc):
                """Prefetch the a2a slices for outproj row chunk rc."""
                sl = slpool.tile([128, KC * 128], BF, tag="sl",
                                 name=f"sl{b}_{rc}")
                nc.sync.dma_start(
                    out=sl[:].rearrange("p (f c) -> p f c", f=KC),
                    in_=a2a_out[b][rc][:, :, :].rearrange("f p c -> p f c"))
                sl_tiles[b, rc] = sl

            def out_rc(b, rc):
                """Output projection for row chunk rc of batch b."""
                sl = sl_tiles.pop((b, rc))
                pso = [psm_pool.tile([128, 512], F32, tag="psm",
                                     name=f"pso{b}_{rc}_{i}")
                       for i in range(2)]
                for fc in range(KC):
                    for n in range(2):
                        nc.tensor.matmul(
                            pso[n][:], sl[:, fc * 128:(fc + 1) * 128],
                            wout_sb[fc][:, n * 512:(n + 1) * 512],
                            start=(fc == 0), stop=(fc == KC - 1))
                ob = obpool.tile([128, DIM], F32, tag="ob",
                                 name=f"ob{b}_{rc}")
                for n in range(2):
                    nc.vector.tensor_tensor(
                        out=ob[:, n * 512:(n + 1) * 512], in0=pso[n][:],
                        in1=bout_sb[:, n * 512:(n + 1) * 512],
                        op=mybir.AluOpType.add)
                nc.sync.dma_start(
                    out=out[b * RPB + rc * 128:b * RPB + (rc + 1) * 128, :],
                    in_=ob[:])

            # ---- filler queue: (kind, cost, fn) PE chunks that hide ----
            # ---- softmax latency inside attention groups            ----
            fillers = []

            def pop_fillers(budget):
                while fillers and fillers[0][1] <= budget:
                    kind, cost, fn = fillers.pop(0)
                    fn()
                    budget -= cost
                    if budget <= 0:
                        break
                return budget

            def drain_qk():
                rest = []
                for u in fillers:
                    if u[0] == "qk":
                        u[2]()
                    else:
                        rest.append(u)
                fillers[:] = rest

            def queue_out(b, rc, rc_too=True):
                fillers.insert(0, ("sl", 0, lambda: out_sl(b, rc)))
                if rc_too:
                    fillers.append(("out", 2, lambda: out_rc(b, rc)))

            def attn_group(b, g, startup=False):
                """Scores+softmax+values for 512 queries, both heads."""
                qT, kT = qk_tiles[b]
                q0 = g * 512
                pts = [None] * NKC
                pavs = None
                budget = [0 if startup else 3]
                av_lag = NKC if startup else LAG

                def av(kc):
                    for h in range(HPC):
                        nc.tensor.matmul(
                            pavs[h][:], ctx_sb[b, h][:, kc * CW:(kc + 1) * CW],
                            pts[kc][:, h * 512:(h + 1) * 512],
                            start=(kc == 0), stop=(kc == NKC - 1))

                for kcb in range(0, NKC, 2):
                    if startup and kcb % 4 == 0 and kcb > 0:
                        t = kcb // 4
                        qk_half(b, t, 1)
                        qk_half(b, t, 0)
                    # two score pairs back-to-back, then their exps, then
                    # a block of four value matmuls: fewer PE mode switches
                    pss = []
                    for kc in (kcb, kcb + 1):
                        ps = pss_pool.tile([128, 1024], F32, tag="pss",
                                           name=f"pss{b}{g}{kc}")
                        pss.append(ps)
                        for h in range(HPC):
                            nc.tensor.matmul(
                                ps[:, h * 512:(h + 1) * 512],
                                kT[h * HD:(h + 1) * HD,
                                   kc * 128:(kc + 1) * 128],
                                qT[h * HD:(h + 1) * HD, q0:q0 + 512],
                                start=True, stop=True,
                                tile_position=(h * HD, 0))
                    for kc in (kcb, kcb + 1):
                        ps = pss[kc - kcb]
                        pt = ptpool.tile([128, 1024], BF, tag="pt",
                                         name=f"pt{b}{g}{kc}")
                        pts[kc] = pt
                        if kc in DVE_KC:
                            nc.vector.tensor_scalar(
                                pt[:].bitcast(I16), ps[:], SCH_A, SCH_B,
                                op0=mybir.AluOpType.mult,
                                op1=mybir.AluOpType.add)
                        else:
                            nc.scalar.activation(
                                pt[:], ps[:],
                                mybir.ActivationFunctionType.Exp, scale=SCALE)
                    if kcb == av_lag:
                        pavs = [psm_pool.tile([CW, 512], F32, tag="psm",
                                              name=f"pav{b}{g}{h}")
                                for h in range(HPC)]
                    if FILL and kcb in (4, 10):
                        budget[0] = pop_fillers(budget[0])
                    if kcb >= av_lag:
                        av(kcb - av_lag)
                        av(kcb - av_lag + 1)
                if pavs is None:
                    pavs = [psm_pool.tile([CW, 512], F32, tag="psm",
                                          name=f"pav{b}{g}{h}")
                            for h in range(HPC)]
                for kc in range(NKC - av_lag, NKC):
                    av(kc)
                # normalize both heads: 1/colsum -> partition_broadcast
                # -> multiply; ship to the a2a buffer in one merged DMA
                p = g // 2
                j0 = (q0 - p * 1024) // 128
                for h in range(HPC):
                    sden = r1pool.tile([1, 512], F32, tag="sden",
                                       name=f"sden{b}{g}{h}")
                    # reciprocal_approx_fast mis-reads partition-base-64
                    # APs; stage the denominator row at base 0 first
                    nc.vector.tensor_copy(sden[:], pavs[h][HD:CW, :])
                    r1 = r1pool.tile([1, 512], F32, tag="r1",
                                     name=f"r1{b}{g}{h}")
                    nc.vector.reciprocal_approx_fast(r1[:], sden[:])
                    rb = rbpool.tile([HD, 512], F32, tag="rb",
                                     name=f"rb{b}{g}{h}")
                    nc.gpsimd.partition_broadcast(rb[:], r1[:], channels=HD)
                    ho = hopool.tile([HD, 512], BF, tag="ho",
                                     name=f"ho{b}{g}{h}")
                    nc.vector.tensor_tensor(
                        out=ho[:], in0=pavs[h][0:HD, :], in1=rb[:],
                        op=mybir.AluOpType.mult)
                    nc.gpsimd.dma_start(
                        out=a2a_in[b][p][j0:j0 + 4, h * HD:(h + 1) * HD, :]
                        .rearrange("j p c -> p j c"),
                        in_=ho[:].rearrange("p (j c) -> p j c", j=4))

            def reshard(b, p):
                nc.gpsimd.collective_compute(
                    "AllToAll", mybir.AluOpType.bypass,
                    replica_groups=[list(range(NC))],
                    ins=[a2a_in[b][p].ap().opt()],
                    outs=[a2a_out[b][p].ap().opt()])

            # ---- schedule ----
            prefetch_x(0, fine=True)
            load_ctx(0)
            prefetch_x(1)
            load_ctx(1)
            qk_half(0, 0, 1)
            qk_half(0, 0, 0)

            for b in range(B):
                if b + 2 < B:
                    prefetch_x(b + 2, defer=True)
                    load_ctx(b + 2, defer=True)
                if b == 0:
                    load_out_consts()
                if b + 1 < B:
                    for t in range(4):
                        for fb in (1, 0):
                            fillers.append(
                                ("qk", 1, lambda b_=b + 1, t_=t, f_=fb:
                                 qk_half(b_, t_, f_)))
                if b >= 1:
                    queue_out(b - 1, 1)
                for g in range(NG):
                    flush_bulk(3)
                    attn_group(b, g, startup=(b == 0 and g == 0))
                    if not FILL:
                        pop_fillers(3)
                    if g == 1:
                        reshard(b, 0)
                        queue_out(b, 0, rc_too=(b < B - 1))
                drain_qk()
                flush_bulk(99)
                reshard(b, 1)
            # tail: held-back outproj covers the last collective's latency
            for kind, cost, fn in fillers:
                fn()
            out_rc(B - 1, 0)
            out_sl(B - 1, 1)
            out_rc(B - 1, 1)
    nc.compile()
    return nc


def prep_inputs(x, context, Wqkv, bqkv, Wout, bout):
    """Host-side sharding: returns in_maps for the 8 cores."""
    x = np.asarray(x, np.float32)
    context = np.asarray(context, np.float32)
    Wqkv = np.asarray(Wqkv, np.float32)
    bqkv = np.asarray(bqkv, np.float32)
    Wout = np.asarray(Wout, np.float32)
    bout = np.asarray(bout, np.float32)

    xT = np.ascontiguousarray(x.reshape(BN, DIM).T).astype(BF16)
    woutT = np.ascontiguousarray(Wout.T).astype(BF16)
    boutb = np.broadcast_to(bout, (128, DIM)).astype(np.float32).copy()

    in_maps = []
    for c in range(NC):
        h0 = c * HPC
        wq = Wqkv[h0 * HD:(h0 + HPC) * HD]
        wk = Wqkv[DIM + h0 * HD:DIM + (h0 + HPC) * HD]
        wqkT = np.ascontiguousarray(
            np.concatenate([wq, wk], axis=0).T).astype(BF16)
        bq = np.concatenate([bqkv[h0 * HD:(h0 + HPC) * HD],
                             bqkv[DIM + h0 * HD:DIM + (h0 + HPC) * HD]])
        bq = bq.reshape(2 * 128, 1).astype(np.float32)
        ctxa = np.ones((B, HPC, 128, NKC, CW), np.float32)
        for h in range(HPC):
            g = h0 + h
            arr = context[:, :, g * HD:(g + 1) * HD].reshape(B, NKC, 128, HD)
            ctxa[:, h, :, :, :HD] = arr.transpose(0, 2, 1, 3)
        in_maps.append({
            "xT": xT,
            "wqkT": wqkT,
            "bqk": bq,
            "ctxa": ctxa.reshape(B, HPC, 128, NKC * CW).astype(BF16),
            "woutT": woutT,
            "boutb": boutb,
        })
    return in_maps


_NC_CACHE = None


def _get_nc():
    global _NC_CACHE
    if _NC_CACHE is None:
        _NC_CACHE = build()
    return _NC_CACHE


def run(in_maps, trace=False):
    nc = _get_nc()
    res = run_bass_kernel_spmd(nc, in_maps, core_ids=list(range(NC)),
                               trace=trace)
    # core c's out[b*256:(b+1)*256] = rows [c*128:(c+1)*128] and
    # [1024+c*128:1024+(c+1)*128] of batch b
    full = np.empty((B, N, DIM), np.float32)
    for c in range(NC):
        o = np.asarray(res.results[c]["out"]).reshape(B, 2, 128, DIM)
        for p in range(2):
            full[:, p * 1024 + c * 128:p * 1024 + (c + 1) * 128, :] = o[:, p]
    return full, res


def kernel(x, context, Wqkv, bqkv, Wout, bout):
    in_maps = prep_inputs(x, context, Wqkv, bqkv, Wout, bout)
    out, _ = run(in_maps, trace=False)
    return out
